# revision 21
# baseline (speedup 1.0000x reference)
"""ConvMultiheadAttention Trainium2 kernel (8 NeuronCores).

Sharding: core c = (batch b = c//2) x (head-group hg = c%2, 8 heads each).
Per core:
  - q/k conv1d projections (K=3, same pad) for this core's 512 output
    channels, expressed as PSUM-accumulated bf16 matmuls over x laid out
    [c_in partitions, L free].
  - v conv computed TRANSPOSED (out = [j partitions, channel free]) by
    swapping matmul operands, so no PE transposes are needed to build the
    AV lhsT; column 64 of each head's 65-wide slot holds ones so the
    softmax denominator falls out of the AV matmul. The v conv is split
    into two half-contractions (cc 0-3 across all j-blocks, then cc 4-7)
    holding all 8 PSUM banks in a scoped pool, so the PE chases the
    interleaved wv/xv DMA chunks with minimal idle at kernel start.
  - attention with TRANSPOSED scores sT[j, i] (j on partitions) so the
    key-padding mask folds into the Exp activation's per-partition bias.
  - partial out-conv contracting over this core's 512 attention-output
    channels; the host sums the two partials per batch. Its first output
    block is accumulated except for the last input block during the last
    pair's exp phase (PE filler), closed once o_x is complete.
DMA: HWDGE setup (~630ns) and the DMA engines are shared, serializing
resources - so transfers are batched into few large instructions:
x(q,k) in one DMA, q/k weights per pair in one, per-occ output DMAs.
Host folds: attention scale + q-bias into q-conv weights/bias; k-bias is
dropped (constant per softmax row -> cancels); v-bias and o-bias are
applied on the host after the gather (attention rows sum to 1).
"""

import os
import numpy as np
import ml_dtypes

BF16 = ml_dtypes.bfloat16

B, L, D = 4, 1024, 1024
NH, HD = 16, 64
KW = 3
NCORES = 8
HALF = D // 2  # channels per core half (8 heads)
SCALE = HD ** -0.5
MASK_BIAS = -30000.0

_CACHE = {}


def _build_nc():
    import concourse.bass as bass  # noqa: F401
    import concourse.tile as tile
    from concourse import bacc, mybir

    f32 = mybir.dt.float32
    bf16 = mybir.dt.bfloat16
    Act = mybir.ActivationFunctionType

    nc = bacc.Bacc(
        "TRN2",
        target_bir_lowering=False,
        debug=False,
        enable_asserts=False,
        num_devices=NCORES,
    )

    # ---- DRAM I/O ----
    xqk_d = nc.dram_tensor("xqk", [128, 2, 8, L], bf16, kind="ExternalInput").ap()
    xv_d = nc.dram_tensor("xv", [128, 8, L], bf16, kind="ExternalInput").ap()
    wqk_d = nc.dram_tensor("wqk", [4, 128, 2, KW, 8, 128], bf16,
                           kind="ExternalInput").ap()
    # v weights in transposed-conv layout: [ci, cc, k, c_local]
    wv_d = nc.dram_tensor("wv", [128, 8, KW, 512], bf16, kind="ExternalInput").ap()
    wo_d = nc.dram_tensor("wo", [8, 128, KW, 4, 128], bf16, kind="ExternalInput").ap()
    qjb_d = nc.dram_tensor("qjb", [128, 12], f32, kind="ExternalInput").ap()
    out_d = nc.dram_tensor("out", [8, 128, L], bf16, kind="ExternalOutput").ap()

    with tile.TileContext(nc) as tc:
        with (
            tc.tile_pool(name="singles", bufs=1) as singles,
            tc.tile_pool(name="wpool", bufs=3) as wpool,
            tc.tile_pool(name="qk", bufs=2) as qkpool,
            tc.tile_pool(name="ppool", bufs=2) as ppool,
            tc.tile_pool(name="outp", bufs=4) as outp,
            tc.tile_pool(name="smalls", bufs=4) as smalls,
        ):
            # ---- resident tiles; x on SP queue, weights on ACT queue,
            #      wv/xv in interleaved 2-cc chunks for DMA chasing ----
            qjb_s = singles.tile([128, 12], f32, tag="qjb")
            xqk_s = singles.tile([128, 2, 8, L], bf16, tag="xqk")
            # xv zero-padded by one column on each side so the transposed
            # v-conv's shifted taps stay full-width (PE psum writes must
            # start at partition 0).
            xv_s = singles.tile([128, 8, L + 2], bf16, tag="xv")
            wv_s = singles.tile([128, 8, KW, 512], bf16, tag="wv")
            nc.vector.memset(xv_s[:, :, 0:1], 0.0)
            nc.vector.memset(xv_s[:, :, L + 1 : L + 2], 0.0)
            # progressive chunks: tiny first so the PE starts ~3us in
            for c0, c1 in ((0, 1), (1, 2), (2, 4), (4, 8)):
                nc.scalar.dma_start(wv_s[:, c0:c1, :, :], wv_d[:, c0:c1, :, :])
                nc.sync.dma_start(xv_s[:, c0:c1, 1 : L + 1], xv_d[:, c0:c1, :])
            nc.sync.dma_start(qjb_s, qjb_d)
            nc.sync.dma_start(xqk_s, xqk_d)

            # vT[j_part, j_chunk, head, 0:64] = v[h*64+d, j]; col 64 = ones
            vT = singles.tile([128, 8, 8, 65], bf16, tag="vT")
            for h in range(8):
                nc.vector.memset(vT[:, :, h, 64:65], 1.0)
            o_x = singles.tile([128, 4, L], bf16, tag="ox")

            def conv_mms(ps, wsel, xsel, ccs, lh, start, stop):
                """Accumulate conv-as-matmul into psum ps[:, 0:512] for
                output columns [lh*512, lh*512+512) over blocks ccs.
                wsel(k, cc) / xsel(cc, a, b) yield the operand APs."""
                # center tap first: full-width start=True write covers the
                # whole bank, so the edge taps' partial-width writes are
                # pure accumulations (uniform has_written state).
                items = [(k, cc) for k in (1, 0, 2) for cc in ccs]
                for idx, (k, cc) in enumerate(items):
                    lo = lh * 512 + k - 1
                    lhsT = wsel(k, cc)
                    if lo < 0:
                        rhs = xsel(cc, 0, 511)
                        outap = ps[:, 1:512]
                    elif lo + 512 > L:
                        rhs = xsel(cc, lo, L)
                        outap = ps[:, 0 : L - lo]
                    else:
                        rhs = xsel(cc, lo, lo + 512)
                        outap = ps[:, 0:512]
                    nc.tensor.matmul(
                        outap,
                        lhsT,
                        rhs,
                        start=(start and idx == 0),
                        stop=(stop and idx == len(items) - 1),
                    )

            def xq_sel(cc, a, b):
                return xqk_s[:, 0, cc, a:b]

            def xk_sel(cc, a, b):
                return xqk_s[:, 1, cc, a:b]

            def ox_sel(cc, a, b):
                return o_x[:, cc, a:b]

            # ---- per head-pair q/k conv units (weights prefetched) ----
            def qk_conv_units(t):
                """Return (q_t, k_t, units): four closures each emitting one
                conv psum-group (~5us of dense PE work) for pair t. The
                combined q+k weight DMA is issued immediately (ACT queue)."""
                q_t = qkpool.tile([128, L], bf16, tag="q", name=f"q{t}")
                k_t = qkpool.tile([128, L], bf16, tag="k", name=f"k{t}")
                wqk_t = wpool.tile([128, 2, KW, 8, 128], bf16, tag="w",
                                   name="wqkt")
                nc.scalar.dma_start(wqk_t, wqk_d[t])

                def unit(which, lh):
                    ps = convp.tile([128, 512], f32, tag="cp")
                    if which == "q":
                        conv_mms(ps, lambda k, cc: wqk_t[:, 0, k, cc, :],
                                 xq_sel, range(8), lh, True, True)
                        nc.vector.tensor_scalar_add(
                            q_t[:, lh * 512 : (lh + 1) * 512], ps,
                            qjb_s[:, t : t + 1])
                    else:
                        conv_mms(ps, lambda k, cc: wqk_t[:, 1, k, cc, :],
                                 xk_sel, range(8), lh, True, True)
                        nc.vector.tensor_copy(
                            k_t[:, lh * 512 : (lh + 1) * 512], ps)

                units = [lambda w=w, lh=lh: unit(w, lh)
                         for w in ("q", "k") for lh in range(2)]
                return q_t, k_t, units

            # issue wqk0 DMA now (behind wv on the ACT queue)
            q_t, k_t, units0 = qk_conv_units(0)

            # ---- V conv, transposed: psum[j, c] = sum_{ci,k} x[ci, j+k-1]
            #      * wv[ci, k, c]. Two half-contraction passes over the
            #      j-blocks (6 banks in a scoped pool, jb6/7 recycle the
            #      first two), so the first pass only needs the first half
            #      of wv/xv and the PE chases the DMA chunks. convp sits
            #      below the vpsum range so the t=0 q/k conv does not wait
            #      on the v-conv drain.
            with (
                tc.tile_pool(name="convp", bufs=2, space="PSUM") as convp,
            ):
                vpsum_cm = tc.tile_pool(name="vpsum", bufs=6, space="PSUM")
                vpsum = vpsum_cm.__enter__()
                vps = {}

                def v_part1(jb):
                    ps = vpsum.tile([128, 512], f32, tag="vp", name=f"vp{jb}")
                    vps[jb] = ps
                    J = jb * 128
                    items = [(k, cc) for cc in range(4) for k in (1, 0, 2)]
                    for idx, (k, cc) in enumerate(items):
                        nc.tensor.matmul(
                            ps,
                            xv_s[:, cc, J + k : J + k + 128],
                            wv_s[:, cc, k, :],
                            start=(idx == 0),
                            stop=False,
                        )

                def v_part2(jb):
                    ps = vps[jb]
                    J = jb * 128
                    items = [(k, cc) for cc in range(4, 8) for k in (1, 0, 2)]
                    for idx, (k, cc) in enumerate(items):
                        nc.tensor.matmul(
                            ps,
                            xv_s[:, cc, J + k : J + k + 128],
                            wv_s[:, cc, k, :],
                            start=False,
                            stop=(idx == len(items) - 1),
                        )
                    nc.scalar.copy(vT[:, jb, 0:8, 0:64], ps[:, 0:512])

                for jb in range(6):
                    v_part1(jb)
                v_part2(0)
                v_part2(1)
                v_part1(6)
                v_part1(7)
                for jb in range(2, 8):
                    v_part2(jb)
                vpsum_cm.__exit__(None, None, None)

                avp_cm = tc.tile_pool(name="avp", bufs=2, space="PSUM")
                avp = avp_cm.__enter__()
                scorep_cm = tc.tile_pool(name="scorep", bufs=2, space="PSUM")
                scorep = scorep_cm.__enter__()
                # q/k conv for t=0 (weights already resident)
                for u in units0:
                    u()

                # ---- o-conv: partial groups (cc 0..2) opened during t=3's
                #      exp phase, closed (cc 3) after; rest run normally ----
                o_state = {}

                def oconv_open_units():
                    for occ in (0, 1):
                        wo_t = wpool.tile([128, KW, 4, 128], bf16, tag="wo",
                                          name=f"wo{occ}")
                        nc.scalar.dma_start(wo_t, wo_d[occ])
                        o_state[f"wo{occ}"] = wo_t

                    def openu(lh):
                        wo_t = o_state["wo0"]
                        ps = convp.tile([128, 512], f32, tag="cp",
                                        name=f"ops{lh}")
                        conv_mms(ps, lambda k, cc: wo_t[:, k, cc, :],
                                 ox_sel, (0, 1, 2), lh, True, False)
                        o_state[f"ps{lh}"] = ps

                    return [lambda lh=lh: openu(lh) for lh in range(2)]

                # ---- per head-pair: software-pipelined q/k conv + attention
                # Per pair t: scores+exp(t) -> q/k conv(t+1) [PE work that
                # hides exp(t) on ACT] -> AV+normalize(t).
                for t in range(4):
                    if t < 3:
                        nq_t, nk_t, conv_units = qk_conv_units(t + 1)
                    else:
                        conv_units = oconv_open_units()
                    # scores + exp for both heads; adjacent matmuls of the two
                    # heads hit disjoint PE row groups (base 0/64) and overlap.
                    p_pair = []
                    for jc in range(8):
                        sps_pair = [scorep.tile([128, L], f32, tag="score",
                                                name=f"sps{hh2}")
                                    for hh2 in range(2)]
                        if jc == 0:
                            p_pair = [ppool.tile([128, 8, L], bf16, tag="p",
                                                 name=f"p{hh2}")
                                      for hh2 in range(2)]
                        for ih in range(2):
                            for hh in range(2):
                                base = hh * 64
                                nc.tensor.matmul(
                                    sps_pair[hh][:, ih * 512 : (ih + 1) * 512],
                                    k_t[base : base + 64,
                                        jc * 128 : (jc + 1) * 128],
                                    q_t[base : base + 64,
                                        ih * 512 : (ih + 1) * 512],
                                    start=True,
                                    stop=True,
                                )
                        for hh in range(2):
                            nc.scalar.activation(
                                p_pair[hh][:, jc, :], sps_pair[hh], Act.Exp,
                                bias=qjb_s[:, 4 + jc : 5 + jc],
                            )
                        # one conv psum-group (~5us dense PE) after every
                        # other jc-group: fills the PE stall while ACT drains
                        if jc % 2 == 1 and conv_units:
                            conv_units.pop(0)()
                    # AV + normalize for both heads
                    for hh in range(2):
                        h = 2 * t + hh
                        base = hh * 64
                        tmp = None
                        if hh == 1:
                            tmp = smalls.tile([64, L], bf16, tag="tmp")
                        for ih in range(2):
                            avps = avp.tile([65, 512], f32, tag="av")
                            for jc in range(8):
                                nc.tensor.matmul(
                                    avps,
                                    vT[:, jc, h, :],
                                    p_pair[hh][:, jc,
                                               ih * 512 : (ih + 1) * 512],
                                    start=(jc == 0),
                                    stop=(jc == 7),
                                )
                            r_t = smalls.tile([1, 512], f32, tag="r")
                            nc.vector.reciprocal(r_t, avps[64:65, :])
                            bc_t = smalls.tile([64, 512], f32, tag="bc")
                            nc.gpsimd.partition_broadcast(bc_t, r_t)
                            if hh == 0:
                                dst = o_x[base : base + 64, t,
                                          ih * 512 : (ih + 1) * 512]
                                nc.vector.tensor_mul(dst, avps[0:64, :], bc_t)
                            else:
                                nc.vector.tensor_mul(
                                    tmp[:, ih * 512 : (ih + 1) * 512],
                                    avps[0:64, :], bc_t)
                                if t == 3:
                                    nc.sync.dma_start(
                                        o_x[64:128, t,
                                            ih * 512 : (ih + 1) * 512],
                                        tmp[:, ih * 512 : (ih + 1) * 512])
                        if hh == 1 and t < 3:
                            nc.sync.dma_start(o_x[64:128, t, :], tmp)
                    if t < 3:
                        q_t, k_t = nq_t, nk_t

                # ---- out conv (partial over this core's 512 in-channels) ---
                # scorep's 4 banks are free now; open a 4-buf o-conv pool and
                # front-load occ1/occ2 partial groups (independent of pair 3)
                # so the PE has work while the last normalize chain drains.
                scorep_cm.__exit__(None, None, None)
                with tc.tile_pool(name="oop", bufs=4, space="PSUM") as oop:
                    wo2 = wpool.tile([128, KW, 4, 128], bf16, tag="wo",
                                     name="wo2")
                    nc.scalar.dma_start(wo2, wo_d[2])
                    o_state["wo2"] = wo2
                    for occ in (1, 2):
                        w_t = o_state[f"wo{occ}"]
                        for lh in range(2):
                            ps = oop.tile([128, 512], f32, tag="op",
                                          name=f"op{occ}{lh}")
                            conv_mms(ps, lambda k, cc, w=w_t: w[:, k, cc, :],
                                     ox_sel, (0, 1, 2), lh, True, False)
                            o_state[f"ps{occ}{lh}"] = ps

                    def o_close(occ, pool_ps):
                        w_t = o_state[f"wo{occ}"]
                        o_t = outp.tile([128, L], bf16, tag="osb")
                        for lh in range(2):
                            ps = pool_ps[lh]
                            conv_mms(ps, lambda k, cc, w=w_t: w[:, k, cc, :],
                                     ox_sel, (3,), lh, False, True)
                            nc.vector.tensor_copy(
                                o_t[:, lh * 512 : (lh + 1) * 512], ps)
                        nc.sync.dma_start(out_d[occ], o_t)

                    wo_next = wpool.tile([128, KW, 4, 128], bf16, tag="wo",
                                         name="wo3")
                    nc.scalar.dma_start(wo_next, wo_d[3])
                    o_close(0, [o_state["ps0"], o_state["ps1"]])
                    o_close(1, [o_state["ps10"], o_state["ps11"]])
                    o_close(2, [o_state["ps20"], o_state["ps21"]])
                    for occ in range(3, 8):
                        wo_t = wo_next
                        if occ < 7:
                            wo_next = wpool.tile([128, KW, 4, 128], bf16,
                                                 tag="wo", name="wo")
                            nc.scalar.dma_start(wo_next, wo_d[occ + 1])
                        o_t = outp.tile([128, L], bf16, tag="osb")
                        if occ < 7:
                            for lh in range(2):
                                ps = oop.tile([128, 512], f32, tag="op")
                                conv_mms(ps,
                                         lambda k, cc, w=wo_t: w[:, k, cc, :],
                                         ox_sel, (0, 1, 2, 3), lh, True, True)
                                nc.vector.tensor_copy(
                                    o_t[:, lh * 512 : (lh + 1) * 512], ps)
                            nc.sync.dma_start(out_d[occ], o_t)
                        else:
                            # quarter-granularity epilogue: each piece's
                            # copy+store overlaps the next piece's matmuls,
                            # so the exposed tail is one short chain
                            for a in (0, 512, 768):
                                w_i = 512 if a == 0 else 256
                                ps = oop.tile([128, 512], f32, tag="op")
                                items = [(k, cc) for k in (1, 0, 2)
                                         for cc in range(4)]
                                for idx, (k, cc) in enumerate(items):
                                    lo = a + k - 1
                                    o0 = max(0, -lo)
                                    o1 = w_i - max(0, lo + w_i - L)
                                    nc.tensor.matmul(
                                        ps[:, o0:o1],
                                        wo_t[:, k, cc, :],
                                        o_x[:, cc, lo + o0 : lo + o1],
                                        start=(idx == 0),
                                        stop=(idx == len(items) - 1),
                                    )
                                nc.vector.tensor_copy(
                                    o_t[:, a : a + w_i], ps[:, 0:w_i])
                                nc.sync.dma_start(
                                    out_d[occ, :, a : a + w_i],
                                    o_t[:, a : a + w_i])
                avp_cm.__exit__(None, None, None)

    nc.compile()
    return nc


def _get_nc():
    if "nc" not in _CACHE:
        _CACHE["nc"] = _build_nc()
    return _CACHE["nc"]


def _prep_inputs(query, key, value, key_padding_mask, attn_mask,
                 q_w, q_b, k_w, k_b, v_w, v_b, o_w, o_b):
    """Build the 8 per-core input maps (host-side shard + layout)."""
    query = np.asarray(query, np.float32)
    key = np.asarray(key, np.float32)
    value = np.asarray(value, np.float32)
    kpm = np.asarray(key_padding_mask)
    attn_mask = np.asarray(attn_mask, np.float32)
    q_w = np.asarray(q_w, np.float32); q_b = np.asarray(q_b, np.float32)
    k_w = np.asarray(k_w, np.float32)
    v_w = np.asarray(v_w, np.float32)
    o_w = np.asarray(o_w, np.float32); o_b = np.asarray(o_b, np.float32)

    # attn_mask must be constant across query rows to fold into the key bias
    if not np.all(attn_mask == attn_mask[0:1, :]):
        raise NotImplementedError("attn_mask varying over query index unsupported")
    am_row = attn_mask[0]

    def conv_w_layout(w, occ, n_ci):
        # w: [C_out_part, C_in_part, KW] -> [occ, p(ci), k, cc, m(c_out)]
        co, ci, _ = w.shape
        arr = w.reshape(occ, 128, n_ci, 128, KW).transpose(0, 3, 4, 2, 1)
        return np.ascontiguousarray(arr).astype(BF16)

    wqk_h, wv_h, wo_h, qjb_q = [], [], [], []
    for hg in range(2):
        sl = slice(hg * HALF, (hg + 1) * HALF)
        wq = conv_w_layout(q_w[sl] * SCALE, 4, 8)
        wk = conv_w_layout(k_w[sl], 4, 8)
        wqk_h.append(np.ascontiguousarray(np.stack([wq, wk], axis=2)))
        # transposed-v layout: [ci, cc, k, c_local]
        wv_h.append(np.ascontiguousarray(
            v_w[sl].transpose(1, 2, 0).reshape(8, 128, KW, 512)
            .transpose(1, 0, 2, 3)).astype(BF16))
        # out conv: contract over this half's input channels
        wo_h.append(conv_w_layout(o_w[:, sl, :], 8, 4))
        qjb_q.append((q_b[sl] * SCALE).reshape(4, 128).T)

    xqk_b, xv_b, jb_b = [], [], []
    for b in range(B):
        xq = query[b].T.reshape(8, 128, L).transpose(1, 0, 2)
        xk = key[b].T.reshape(8, 128, L).transpose(1, 0, 2)
        xqk_b.append(np.ascontiguousarray(
            np.stack([xq, xk], axis=1)).astype(BF16))
        xv_b.append(np.ascontiguousarray(
            value[b].T.reshape(8, 128, L).transpose(1, 0, 2)).astype(BF16))
        jb = np.where(kpm[b], MASK_BIAS, 0.0).astype(np.float32) + am_row
        jb_b.append(jb.reshape(8, 128).T)

    in_maps = []
    for c in range(NCORES):
        b, hg = c // 2, c % 2
        qjb = np.ascontiguousarray(
            np.concatenate([qjb_q[hg], jb_b[b]], axis=1)).astype(np.float32)
        in_maps.append({
            "xqk": xqk_b[b], "xv": xv_b[b],
            "wqk": wqk_h[hg], "wv": wv_h[hg], "wo": wo_h[hg],
            "qjb": qjb,
        })
    return in_maps, (o_w, np.asarray(v_b, np.float32), o_b)


def _postprocess(parts, extras):
    """parts: list of 8 arrays [8,128,L] bf16 -> full output [B, L, D] f32."""
    o_w, v_b, o_b = extras
    # v-bias contribution through the out conv (attention rows sum to 1):
    # interior columns see all 3 taps, edge columns lose one.
    a_full = o_w.sum(axis=2) @ v_b            # [D]
    a_l0 = a_full - o_w[:, :, 0] @ v_b        # l = 0 loses tap k=0
    a_lL = a_full - o_w[:, :, 2] @ v_b        # l = L-1 loses tap k=2
    out = np.empty((B, L, D), np.float32)
    for b in range(B):
        tot = (parts[2 * b].astype(np.float32)
               + parts[2 * b + 1].astype(np.float32)).reshape(D, L)
        tot = tot + o_b[:, None] + a_full[:, None]
        tot[:, 0] += a_l0 - a_full
        tot[:, -1] += a_lL - a_full
        out[b] = tot.T
    return out


def _run(in_maps, trace=False, **kw):
    from concourse import bass_utils
    nc = _get_nc()
    try:
        res = bass_utils.run_bass_kernel_spmd(
            nc, in_maps, core_ids=list(range(NCORES)), trace=trace, **kw)
    except ModuleNotFoundError:
        # NTFF profiling hook unavailable (axon client without axon.trn);
        # rerun without trace.
        res = bass_utils.run_bass_kernel_spmd(
            nc, in_maps, core_ids=list(range(NCORES)), trace=False, **kw)
    return res


def kernel(**inputs) -> np.ndarray:
    in_maps, extras = _prep_inputs(**inputs)
    res = _run(in_maps, trace=bool(int(os.environ.get("KERNEL_TRACE", "0"))))
    parts = [res.results[c]["out"] for c in range(NCORES)]
    out = _postprocess(parts, extras)
    if res.exec_time_ns is not None:
        print(f"HW exec time: {res.exec_time_ns} ns")
    return out


# revision 22
# speedup vs baseline: 1.0006x; 1.0006x over previous
"""ConvMultiheadAttention Trainium2 kernel (8 NeuronCores).

Sharding: core c = (batch b = c//2) x (head-group hg = c%2, 8 heads each).
Per core:
  - q/k conv1d projections (K=3, same pad) for this core's 512 output
    channels, expressed as PSUM-accumulated bf16 matmuls over x laid out
    [c_in partitions, L free].
  - v conv computed TRANSPOSED (out = [j partitions, channel free]) by
    swapping matmul operands, so no PE transposes are needed to build the
    AV lhsT; column 64 of each head's 65-wide slot holds ones so the
    softmax denominator falls out of the AV matmul. The v conv is split
    into two half-contractions (cc 0-3 across all j-blocks, then cc 4-7)
    holding all 8 PSUM banks in a scoped pool, so the PE chases the
    interleaved wv/xv DMA chunks with minimal idle at kernel start.
  - attention with TRANSPOSED scores sT[j, i] (j on partitions) so the
    key-padding mask folds into the Exp activation's per-partition bias.
  - partial out-conv contracting over this core's 512 attention-output
    channels; the host sums the two partials per batch. Its first output
    block is accumulated except for the last input block during the last
    pair's exp phase (PE filler), closed once o_x is complete.
DMA: HWDGE setup (~630ns) and the DMA engines are shared, serializing
resources - so transfers are batched into few large instructions:
x(q,k) in one DMA, q/k weights per pair in one, per-occ output DMAs.
Host folds: attention scale + q-bias into q-conv weights/bias; k-bias is
dropped (constant per softmax row -> cancels); v-bias and o-bias are
applied on the host after the gather (attention rows sum to 1).
"""

import os
import numpy as np
import ml_dtypes

BF16 = ml_dtypes.bfloat16

B, L, D = 4, 1024, 1024
NH, HD = 16, 64
KW = 3
NCORES = 8
HALF = D // 2  # channels per core half (8 heads)
SCALE = HD ** -0.5
MASK_BIAS = -30000.0

_CACHE = {}


def _build_nc():
    import concourse.bass as bass  # noqa: F401
    import concourse.tile as tile
    from concourse import bacc, mybir

    f32 = mybir.dt.float32
    bf16 = mybir.dt.bfloat16
    Act = mybir.ActivationFunctionType

    nc = bacc.Bacc(
        "TRN2",
        target_bir_lowering=False,
        debug=False,
        enable_asserts=False,
        num_devices=NCORES,
    )

    # ---- DRAM I/O ----
    xqk_d = nc.dram_tensor("xqk", [128, 2, 8, L], bf16, kind="ExternalInput").ap()
    xv_d = nc.dram_tensor("xv", [128, 8, L], bf16, kind="ExternalInput").ap()
    wqk_d = nc.dram_tensor("wqk", [4, 128, 2, KW, 8, 128], bf16,
                           kind="ExternalInput").ap()
    # v weights in transposed-conv layout: [ci, cc, k, c_local]
    wv_d = nc.dram_tensor("wv", [128, 8, KW, 512], bf16, kind="ExternalInput").ap()
    wo_d = nc.dram_tensor("wo", [8, 128, KW, 4, 128], bf16, kind="ExternalInput").ap()
    qjb_d = nc.dram_tensor("qjb", [128, 12], f32, kind="ExternalInput").ap()
    out_d = nc.dram_tensor("out", [8, 128, L], bf16, kind="ExternalOutput").ap()

    with tile.TileContext(nc) as tc:
        with (
            tc.tile_pool(name="singles", bufs=1) as singles,
            tc.tile_pool(name="wpool", bufs=3) as wpool,
            tc.tile_pool(name="qk", bufs=2) as qkpool,
            tc.tile_pool(name="ppool", bufs=2) as ppool,
            tc.tile_pool(name="outp", bufs=4) as outp,
            tc.tile_pool(name="smalls", bufs=4) as smalls,
        ):
            # ---- resident tiles; x on SP queue, weights on ACT queue,
            #      wv/xv in interleaved 2-cc chunks for DMA chasing ----
            qjb_s = singles.tile([128, 12], f32, tag="qjb")
            xqk_s = singles.tile([128, 2, 8, L], bf16, tag="xqk")
            # xv zero-padded by one column on each side so the transposed
            # v-conv's shifted taps stay full-width (PE psum writes must
            # start at partition 0).
            xv_s = singles.tile([128, 8, L + 2], bf16, tag="xv")
            wv_s = singles.tile([128, 8, KW, 512], bf16, tag="wv")
            nc.vector.memset(xv_s[:, :, 0:1], 0.0)
            nc.vector.memset(xv_s[:, :, L + 1 : L + 2], 0.0)
            # progressive chunks: tiny first so the PE starts ~3us in
            for c0, c1 in ((0, 1), (1, 2), (2, 4), (4, 8)):
                nc.scalar.dma_start(wv_s[:, c0:c1, :, :], wv_d[:, c0:c1, :, :])
                nc.sync.dma_start(xv_s[:, c0:c1, 1 : L + 1], xv_d[:, c0:c1, :])
            nc.sync.dma_start(qjb_s, qjb_d)
            nc.sync.dma_start(xqk_s, xqk_d)

            # vT[j_part, j_chunk, head, 0:64] = v[h*64+d, j]; col 64 = ones
            vT = singles.tile([128, 8, 8, 65], bf16, tag="vT")
            for h in range(8):
                nc.vector.memset(vT[:, :, h, 64:65], 1.0)
            o_x = singles.tile([128, 4, L], bf16, tag="ox")

            def conv_mms(ps, wsel, xsel, ccs, lh, start, stop):
                """Accumulate conv-as-matmul into psum ps[:, 0:512] for
                output columns [lh*512, lh*512+512) over blocks ccs.
                wsel(k, cc) / xsel(cc, a, b) yield the operand APs."""
                # center tap first: full-width start=True write covers the
                # whole bank, so the edge taps' partial-width writes are
                # pure accumulations (uniform has_written state).
                items = [(k, cc) for k in (1, 0, 2) for cc in ccs]
                for idx, (k, cc) in enumerate(items):
                    lo = lh * 512 + k - 1
                    lhsT = wsel(k, cc)
                    if lo < 0:
                        rhs = xsel(cc, 0, 511)
                        outap = ps[:, 1:512]
                    elif lo + 512 > L:
                        rhs = xsel(cc, lo, L)
                        outap = ps[:, 0 : L - lo]
                    else:
                        rhs = xsel(cc, lo, lo + 512)
                        outap = ps[:, 0:512]
                    nc.tensor.matmul(
                        outap,
                        lhsT,
                        rhs,
                        start=(start and idx == 0),
                        stop=(stop and idx == len(items) - 1),
                    )

            def xq_sel(cc, a, b):
                return xqk_s[:, 0, cc, a:b]

            def xk_sel(cc, a, b):
                return xqk_s[:, 1, cc, a:b]

            def ox_sel(cc, a, b):
                return o_x[:, cc, a:b]

            # ---- per head-pair q/k conv units (weights prefetched) ----
            def qk_conv_units(t):
                """Return (q_t, k_t, units): four closures each emitting one
                conv psum-group (~5us of dense PE work) for pair t. The
                combined q+k weight DMA is issued immediately (ACT queue)."""
                q_t = qkpool.tile([128, L], bf16, tag="q", name=f"q{t}")
                k_t = qkpool.tile([128, L], bf16, tag="k", name=f"k{t}")
                wqk_t = wpool.tile([128, 2, KW, 8, 128], bf16, tag="w",
                                   name="wqkt")
                nc.scalar.dma_start(wqk_t, wqk_d[t])

                def unit(which, lh):
                    ps = convp.tile([128, 512], f32, tag="cp")
                    if which == "q":
                        conv_mms(ps, lambda k, cc: wqk_t[:, 0, k, cc, :],
                                 xq_sel, range(8), lh, True, True)
                        nc.vector.tensor_scalar_add(
                            q_t[:, lh * 512 : (lh + 1) * 512], ps,
                            qjb_s[:, t : t + 1])
                    else:
                        conv_mms(ps, lambda k, cc: wqk_t[:, 1, k, cc, :],
                                 xk_sel, range(8), lh, True, True)
                        nc.vector.tensor_copy(
                            k_t[:, lh * 512 : (lh + 1) * 512], ps)

                units = [lambda w=w, lh=lh: unit(w, lh)
                         for w in ("q", "k") for lh in range(2)]
                return q_t, k_t, units

            # issue wqk0 DMA now (behind wv on the ACT queue)
            q_t, k_t, units0 = qk_conv_units(0)

            # ---- V conv, transposed: psum[j, c] = sum_{ci,k} x[ci, j+k-1]
            #      * wv[ci, k, c]. Two half-contraction passes over the
            #      j-blocks (6 banks in a scoped pool, jb6/7 recycle the
            #      first two), so the first pass only needs the first half
            #      of wv/xv and the PE chases the DMA chunks. convp sits
            #      below the vpsum range so the t=0 q/k conv does not wait
            #      on the v-conv drain.
            with (
                tc.tile_pool(name="convp", bufs=2, space="PSUM") as convp,
            ):
                vpsum_cm = tc.tile_pool(name="vpsum", bufs=6, space="PSUM")
                vpsum = vpsum_cm.__enter__()
                vps = {}

                def v_part1(jb):
                    ps = vpsum.tile([128, 512], f32, tag="vp", name=f"vp{jb}")
                    vps[jb] = ps
                    J = jb * 128
                    items = [(k, cc) for cc in range(4) for k in (1, 0, 2)]
                    for idx, (k, cc) in enumerate(items):
                        nc.tensor.matmul(
                            ps,
                            xv_s[:, cc, J + k : J + k + 128],
                            wv_s[:, cc, k, :],
                            start=(idx == 0),
                            stop=False,
                        )

                def v_part2(jb):
                    ps = vps[jb]
                    J = jb * 128
                    items = [(k, cc) for cc in range(4, 8) for k in (1, 0, 2)]
                    for idx, (k, cc) in enumerate(items):
                        nc.tensor.matmul(
                            ps,
                            xv_s[:, cc, J + k : J + k + 128],
                            wv_s[:, cc, k, :],
                            start=False,
                            stop=(idx == len(items) - 1),
                        )
                    nc.vector.tensor_copy(vT[:, jb, 0:8, 0:64], ps[:, 0:512])

                for jb in range(6):
                    v_part1(jb)
                v_part2(0)
                v_part2(1)
                v_part1(6)
                v_part1(7)
                for jb in range(2, 8):
                    v_part2(jb)
                vpsum_cm.__exit__(None, None, None)

                avp_cm = tc.tile_pool(name="avp", bufs=2, space="PSUM")
                avp = avp_cm.__enter__()
                scorep_cm = tc.tile_pool(name="scorep", bufs=2, space="PSUM")
                scorep = scorep_cm.__enter__()
                # q/k conv for t=0 (weights already resident)
                for u in units0:
                    u()

                # ---- o-conv: partial groups (cc 0..2) opened during t=3's
                #      exp phase, closed (cc 3) after; rest run normally ----
                o_state = {}

                def oconv_open_units():
                    for occ in (0, 1):
                        wo_t = wpool.tile([128, KW, 4, 128], bf16, tag="wo",
                                          name=f"wo{occ}")
                        nc.scalar.dma_start(wo_t, wo_d[occ])
                        o_state[f"wo{occ}"] = wo_t

                    def openu(lh):
                        wo_t = o_state["wo0"]
                        ps = convp.tile([128, 512], f32, tag="cp",
                                        name=f"ops{lh}")
                        conv_mms(ps, lambda k, cc: wo_t[:, k, cc, :],
                                 ox_sel, (0, 1, 2), lh, True, False)
                        o_state[f"ps{lh}"] = ps

                    return [lambda lh=lh: openu(lh) for lh in range(2)]

                # ---- per head-pair: software-pipelined q/k conv + attention
                # Per pair t: scores+exp(t) -> q/k conv(t+1) [PE work that
                # hides exp(t) on ACT] -> AV+normalize(t).
                for t in range(4):
                    if t < 3:
                        nq_t, nk_t, conv_units = qk_conv_units(t + 1)
                    else:
                        conv_units = oconv_open_units()
                    # scores + exp for both heads; adjacent matmuls of the two
                    # heads hit disjoint PE row groups (base 0/64) and overlap.
                    p_pair = []
                    for jc in range(8):
                        sps_pair = [scorep.tile([128, L], f32, tag="score",
                                                name=f"sps{hh2}")
                                    for hh2 in range(2)]
                        if jc == 0:
                            p_pair = [ppool.tile([128, 8, L], bf16, tag="p",
                                                 name=f"p{hh2}")
                                      for hh2 in range(2)]
                        for ih in range(2):
                            for hh in range(2):
                                base = hh * 64
                                nc.tensor.matmul(
                                    sps_pair[hh][:, ih * 512 : (ih + 1) * 512],
                                    k_t[base : base + 64,
                                        jc * 128 : (jc + 1) * 128],
                                    q_t[base : base + 64,
                                        ih * 512 : (ih + 1) * 512],
                                    start=True,
                                    stop=True,
                                )
                        for hh in range(2):
                            nc.scalar.activation(
                                p_pair[hh][:, jc, :], sps_pair[hh], Act.Exp,
                                bias=qjb_s[:, 4 + jc : 5 + jc],
                            )
                        # one conv psum-group (~5us dense PE) after every
                        # other jc-group: fills the PE stall while ACT drains
                        if jc % 2 == 1 and conv_units:
                            conv_units.pop(0)()
                    # AV + normalize for both heads
                    for hh in range(2):
                        h = 2 * t + hh
                        base = hh * 64
                        tmp = None
                        if hh == 1:
                            tmp = smalls.tile([64, L], bf16, tag="tmp")
                        for ih in range(2):
                            avps = avp.tile([65, 512], f32, tag="av")
                            for jc in range(8):
                                nc.tensor.matmul(
                                    avps,
                                    vT[:, jc, h, :],
                                    p_pair[hh][:, jc,
                                               ih * 512 : (ih + 1) * 512],
                                    start=(jc == 0),
                                    stop=(jc == 7),
                                )
                            r_t = smalls.tile([1, 512], f32, tag="r")
                            nc.vector.reciprocal(r_t, avps[64:65, :])
                            bc_t = smalls.tile([64, 512], f32, tag="bc")
                            nc.gpsimd.partition_broadcast(bc_t, r_t)
                            if hh == 0:
                                dst = o_x[base : base + 64, t,
                                          ih * 512 : (ih + 1) * 512]
                                nc.vector.tensor_mul(dst, avps[0:64, :], bc_t)
                            else:
                                nc.vector.tensor_mul(
                                    tmp[:, ih * 512 : (ih + 1) * 512],
                                    avps[0:64, :], bc_t)
                                if t == 3:
                                    nc.sync.dma_start(
                                        o_x[64:128, t,
                                            ih * 512 : (ih + 1) * 512],
                                        tmp[:, ih * 512 : (ih + 1) * 512])
                        if hh == 1 and t < 3:
                            nc.sync.dma_start(o_x[64:128, t, :], tmp)
                    if t < 3:
                        q_t, k_t = nq_t, nk_t

                # ---- out conv (partial over this core's 512 in-channels) ---
                # scorep's 4 banks are free now; open a 4-buf o-conv pool and
                # front-load occ1/occ2 partial groups (independent of pair 3)
                # so the PE has work while the last normalize chain drains.
                scorep_cm.__exit__(None, None, None)
                with tc.tile_pool(name="oop", bufs=4, space="PSUM") as oop:
                    wo2 = wpool.tile([128, KW, 4, 128], bf16, tag="wo",
                                     name="wo2")
                    nc.scalar.dma_start(wo2, wo_d[2])
                    o_state["wo2"] = wo2
                    for occ in (1, 2):
                        w_t = o_state[f"wo{occ}"]
                        for lh in range(2):
                            ps = oop.tile([128, 512], f32, tag="op",
                                          name=f"op{occ}{lh}")
                            conv_mms(ps, lambda k, cc, w=w_t: w[:, k, cc, :],
                                     ox_sel, (0, 1, 2), lh, True, False)
                            o_state[f"ps{occ}{lh}"] = ps

                    def o_close(occ, pool_ps):
                        w_t = o_state[f"wo{occ}"]
                        o_t = outp.tile([128, L], bf16, tag="osb")
                        for lh in range(2):
                            ps = pool_ps[lh]
                            conv_mms(ps, lambda k, cc, w=w_t: w[:, k, cc, :],
                                     ox_sel, (3,), lh, False, True)
                            nc.vector.tensor_copy(
                                o_t[:, lh * 512 : (lh + 1) * 512], ps)
                        nc.sync.dma_start(out_d[occ], o_t)

                    wo_next = wpool.tile([128, KW, 4, 128], bf16, tag="wo",
                                         name="wo3")
                    nc.scalar.dma_start(wo_next, wo_d[3])
                    o_close(0, [o_state["ps0"], o_state["ps1"]])
                    o_close(1, [o_state["ps10"], o_state["ps11"]])
                    o_close(2, [o_state["ps20"], o_state["ps21"]])
                    for occ in range(3, 8):
                        wo_t = wo_next
                        if occ < 7:
                            wo_next = wpool.tile([128, KW, 4, 128], bf16,
                                                 tag="wo", name="wo")
                            nc.scalar.dma_start(wo_next, wo_d[occ + 1])
                        o_t = outp.tile([128, L], bf16, tag="osb")
                        if occ < 7:
                            for lh in range(2):
                                ps = oop.tile([128, 512], f32, tag="op")
                                conv_mms(ps,
                                         lambda k, cc, w=wo_t: w[:, k, cc, :],
                                         ox_sel, (0, 1, 2, 3), lh, True, True)
                                nc.vector.tensor_copy(
                                    o_t[:, lh * 512 : (lh + 1) * 512], ps)
                            nc.sync.dma_start(out_d[occ], o_t)
                        else:
                            # quarter-granularity epilogue: each piece's
                            # copy+store overlaps the next piece's matmuls,
                            # so the exposed tail is one short chain
                            for a in (0, 512, 768):
                                w_i = 512 if a == 0 else 256
                                ps = oop.tile([128, 512], f32, tag="op")
                                items = [(k, cc) for k in (1, 0, 2)
                                         for cc in range(4)]
                                for idx, (k, cc) in enumerate(items):
                                    lo = a + k - 1
                                    o0 = max(0, -lo)
                                    o1 = w_i - max(0, lo + w_i - L)
                                    nc.tensor.matmul(
                                        ps[:, o0:o1],
                                        wo_t[:, k, cc, :],
                                        o_x[:, cc, lo + o0 : lo + o1],
                                        start=(idx == 0),
                                        stop=(idx == len(items) - 1),
                                    )
                                nc.vector.tensor_copy(
                                    o_t[:, a : a + w_i], ps[:, 0:w_i])
                                nc.sync.dma_start(
                                    out_d[occ, :, a : a + w_i],
                                    o_t[:, a : a + w_i])
                avp_cm.__exit__(None, None, None)

    nc.compile()
    return nc


def _get_nc():
    if "nc" not in _CACHE:
        _CACHE["nc"] = _build_nc()
    return _CACHE["nc"]


def _prep_inputs(query, key, value, key_padding_mask, attn_mask,
                 q_w, q_b, k_w, k_b, v_w, v_b, o_w, o_b):
    """Build the 8 per-core input maps (host-side shard + layout)."""
    query = np.asarray(query, np.float32)
    key = np.asarray(key, np.float32)
    value = np.asarray(value, np.float32)
    kpm = np.asarray(key_padding_mask)
    attn_mask = np.asarray(attn_mask, np.float32)
    q_w = np.asarray(q_w, np.float32); q_b = np.asarray(q_b, np.float32)
    k_w = np.asarray(k_w, np.float32)
    v_w = np.asarray(v_w, np.float32)
    o_w = np.asarray(o_w, np.float32); o_b = np.asarray(o_b, np.float32)

    # attn_mask must be constant across query rows to fold into the key bias
    if not np.all(attn_mask == attn_mask[0:1, :]):
        raise NotImplementedError("attn_mask varying over query index unsupported")
    am_row = attn_mask[0]

    def conv_w_layout(w, occ, n_ci):
        # w: [C_out_part, C_in_part, KW] -> [occ, p(ci), k, cc, m(c_out)]
        co, ci, _ = w.shape
        arr = w.reshape(occ, 128, n_ci, 128, KW).transpose(0, 3, 4, 2, 1)
        return np.ascontiguousarray(arr).astype(BF16)

    wqk_h, wv_h, wo_h, qjb_q = [], [], [], []
    for hg in range(2):
        sl = slice(hg * HALF, (hg + 1) * HALF)
        wq = conv_w_layout(q_w[sl] * SCALE, 4, 8)
        wk = conv_w_layout(k_w[sl], 4, 8)
        wqk_h.append(np.ascontiguousarray(np.stack([wq, wk], axis=2)))
        # transposed-v layout: [ci, cc, k, c_local]
        wv_h.append(np.ascontiguousarray(
            v_w[sl].transpose(1, 2, 0).reshape(8, 128, KW, 512)
            .transpose(1, 0, 2, 3)).astype(BF16))
        # out conv: contract over this half's input channels
        wo_h.append(conv_w_layout(o_w[:, sl, :], 8, 4))
        qjb_q.append((q_b[sl] * SCALE).reshape(4, 128).T)

    xqk_b, xv_b, jb_b = [], [], []
    for b in range(B):
        xq = query[b].T.reshape(8, 128, L).transpose(1, 0, 2)
        xk = key[b].T.reshape(8, 128, L).transpose(1, 0, 2)
        xqk_b.append(np.ascontiguousarray(
            np.stack([xq, xk], axis=1)).astype(BF16))
        xv_b.append(np.ascontiguousarray(
            value[b].T.reshape(8, 128, L).transpose(1, 0, 2)).astype(BF16))
        jb = np.where(kpm[b], MASK_BIAS, 0.0).astype(np.float32) + am_row
        jb_b.append(jb.reshape(8, 128).T)

    in_maps = []
    for c in range(NCORES):
        b, hg = c // 2, c % 2
        qjb = np.ascontiguousarray(
            np.concatenate([qjb_q[hg], jb_b[b]], axis=1)).astype(np.float32)
        in_maps.append({
            "xqk": xqk_b[b], "xv": xv_b[b],
            "wqk": wqk_h[hg], "wv": wv_h[hg], "wo": wo_h[hg],
            "qjb": qjb,
        })
    return in_maps, (o_w, np.asarray(v_b, np.float32), o_b)


def _postprocess(parts, extras):
    """parts: list of 8 arrays [8,128,L] bf16 -> full output [B, L, D] f32."""
    o_w, v_b, o_b = extras
    # v-bias contribution through the out conv (attention rows sum to 1):
    # interior columns see all 3 taps, edge columns lose one.
    a_full = o_w.sum(axis=2) @ v_b            # [D]
    a_l0 = a_full - o_w[:, :, 0] @ v_b        # l = 0 loses tap k=0
    a_lL = a_full - o_w[:, :, 2] @ v_b        # l = L-1 loses tap k=2
    out = np.empty((B, L, D), np.float32)
    for b in range(B):
        tot = (parts[2 * b].astype(np.float32)
               + parts[2 * b + 1].astype(np.float32)).reshape(D, L)
        tot = tot + o_b[:, None] + a_full[:, None]
        tot[:, 0] += a_l0 - a_full
        tot[:, -1] += a_lL - a_full
        out[b] = tot.T
    return out


def _run(in_maps, trace=False, **kw):
    from concourse import bass_utils
    nc = _get_nc()
    try:
        res = bass_utils.run_bass_kernel_spmd(
            nc, in_maps, core_ids=list(range(NCORES)), trace=trace, **kw)
    except ModuleNotFoundError:
        # NTFF profiling hook unavailable (axon client without axon.trn);
        # rerun without trace.
        res = bass_utils.run_bass_kernel_spmd(
            nc, in_maps, core_ids=list(range(NCORES)), trace=False, **kw)
    return res


def kernel(**inputs) -> np.ndarray:
    in_maps, extras = _prep_inputs(**inputs)
    res = _run(in_maps, trace=bool(int(os.environ.get("KERNEL_TRACE", "0"))))
    parts = [res.results[c]["out"] for c in range(NCORES)]
    out = _postprocess(parts, extras)
    if res.exec_time_ns is not None:
        print(f"HW exec time: {res.exec_time_ns} ns")
    return out
